# revision 11
# baseline (speedup 1.0000x reference)
"""Trainium2 Bass kernel for nn_Attention_41472204210940.

Reference computation (per batch b):
    q = x @ Wq; k, v = split(x @ Wkv); multi-head attention (H=8, DH=64);
    out = attn_out @ Wout + bout.

Sharding over 8 NeuronCores: core c handles batch b = c//2 and head group
g = c%2 (heads 4g..4g+4). Each core emits a partial [2048, 512] output; the
host sums the two partials per batch and adds bout (row-parallel to_out).

Per-core device program:
  - QK^T in bf16 exactly as the tuned baseline: ST[j, i] = K^T Q per
    (head, key-chunk, query-block), softmax transposed so the mask bias is
    a per-partition scalar.
  - softmax exp is SPLIT between engines: most key-chunk pairs go through
    the ACT engine (exp -> fp8e4m3 output), a tunable subset through the
    DVE as a one-instruction Schraudolph exp (int16 = st*A + B, whose bits
    ARE the bf16 of exp; fp32->int16 saturation maps masked lanes to -0.0).
  - PV: for fp8 pairs one DoubleRow fp8 matmul contracts BOTH key chunks
    (K=2x128) in the cycles a bf16 matmul needs for one chunk (hardware
    measured: 260ns either way); V stored fp8 with a ones column per head
    (16B-aligned 80-col head slots per the s3_lw_dual_fp8 stride rule) so
    the softmax denominators fall out of the same matmul. Schraudolph
    pairs run per-chunk matmuls with fp8 stationary x bf16 moving (mixed
    dtypes measured exact on HW).
  - epilogue uses reciprocal_approx_fast (~5x cheaper than DVE reciprocal,
    18 bits) + gpsimd partition-broadcast + DVE multiply into AOT.
  - out[t] = sum_pair AOT_pair[:, t].T @ Wout_pair in bf16, as before.

fp8/Schraudolph noise budget (measured against the cached reference
inputs in numpy, element-exact): ~1.3-1.5e-2 vs the 2e-2 gate.
"""

import numpy as np

B, N, D = 4, 2048, 512
H_TOTAL, DH = 8, 64
HEADS = 4            # heads per core
INNER = HEADS * DH   # per-core inner width (256)
N_CORES = 8
SCALE = DH ** -0.5

# Schraudolph constants: int16(bits of bf16(exp(s))) ~= s*SCH_A + SCH_B
SCH_AU = 12102203.161561485 / 65536.0        # 2^23/ln2 / 2^16, unscaled
SCH_B = 1064866805.0 / 65536.0               # RMS-optimal bias /2^16


def build_program(n=N, d=D, heads=HEADS, dh=DH,
                  inject_v=True, inject_qk1=True, inject_final=True,
                  qk_interleave=True,
                  p8_bufs=6, pi_bufs=3,
                  attn_prio=True, dma_all_sync=True,
                  pre_ot0=True, wqk_scalar=True,
                  early_act_evac=0, warmup_mms=24,
                  dve_pairs=(3, 6), use_dr=True, dbg=False):
    """Build + compile the per-core Bass program (SPMD; all cores run the
    identical program on different data).

    dve_pairs: pair indices (0..NJ/2-1) within every (pass, query-block)
    whose softmax runs on the DVE via Schraudolph instead of ACT exp."""
    import concourse.bacc as bacc
    import concourse.mybir as mybir
    from concourse import tile

    f32 = mybir.dt.float32
    bf = mybir.dt.bfloat16
    f8 = mybir.dt.float8e4
    i16 = mybir.dt.int16
    u8 = mybir.dt.uint8
    AF = mybir.ActivationFunctionType
    Alu = mybir.AluOpType
    DR = mybir.MatmulPerfMode.DoubleRow

    inner = heads * dh
    KC = d // 128          # k-chunks of the projection contraction dim
    IC = inner // 128      # 128-row chunks of QT/KT == head pairs
    NJ = n // 128          # key chunks
    NP = NJ // 2           # key chunk pairs
    NI = n // 512          # query blocks
    VW = dh + 1            # V columns per head incl. the ones column
    VS = 80                # V head slot width (16B-aligned for dual-fp8 LDW)

    assert dh == 64 and inner % 128 == 0 and n % 512 == 0 and d % 128 == 0

    nc = bacc.Bacc("TRN2", target_bir_lowering=False, debug=False)

    xt_d = nc.dram_tensor("xt", [d, n], bf, kind="ExternalInput")
    wq_d = nc.dram_tensor("wq", [d, inner], bf, kind="ExternalInput")
    wk_d = nc.dram_tensor("wk", [d, inner], bf, kind="ExternalInput")
    wv_d = nc.dram_tensor("wv", [d, inner], bf, kind="ExternalInput")
    wo_d = nc.dram_tensor("wo", [inner, d], bf, kind="ExternalInput")
    mask_d = nc.dram_tensor("mask", [n], u8, kind="ExternalInput")
    out_d = nc.dram_tensor("out", [n, d], f32, kind="ExternalOutput")
    if dbg:
        dbg_qt = nc.dram_tensor("dbg_qt", [128, 512], bf, kind="ExternalOutput")
        dbg_kt = nc.dram_tensor("dbg_kt", [128, 512], bf, kind="ExternalOutput")
        dbg_v2 = nc.dram_tensor("dbg_v2", [128, 640], f8, kind="ExternalOutput")
        dbg_p2 = nc.dram_tensor("dbg_p2", [128, 2048], f8, kind="ExternalOutput")
        dbg_ot = nc.dram_tensor("dbg_ot", [VW, 1024], f32, kind="ExternalOutput")
        dbg_rc = nc.dram_tensor("dbg_rc", [1, 512], f32, kind="ExternalOutput")
        dbg_aot = nc.dram_tensor("dbg_aot", [128, 512], bf, kind="ExternalOutput")

    with tile.TileContext(nc) as tc:
        with (
            nc.allow_low_precision(reason="bf16/fp8 matmul operand prep"),
            tc.tile_pool(name="const", bufs=1) as cpool,
            tc.tile_pool(name="p8w", bufs=p8_bufs) as p8pool,
            tc.tile_pool(name="piw", bufs=pi_bufs) as pipool,
            tc.tile_pool(name="small", bufs=2) as spool,
            tc.tile_pool(name="outsb", bufs=3) as opool,
            tc.tile_pool(name="mm", bufs=2, space="PSUM") as mmpool,
            tc.tile_pool(name="ot", bufs=2, space="PSUM") as otpool,
        ):
            # first attention block's OT accumulator comes from the mm
            # pool, allocated before everything else (see baseline notes)
            ot0 = (mmpool.tile([VW, 1024], f32, tag="mm", name="ot0")
                   if pre_ot0 else None)

            # ---- input loads: fused tiles, few DGE issues ----
            xTa = cpool.tile([128, KC * n], bf, name="xTa")
            wqa = cpool.tile([128, KC * inner], bf, name="wqa")
            wka = cpool.tile([128, KC * inner], bf, name="wka")
            wva = cpool.tile([128, KC * inner], bf, name="wva")
            wo = [cpool.tile([128, d], bf, name=f"wo{i}") for i in range(IC)]

            def xT(k):
                return xTa[:, n * k:n * (k + 1)]

            def wslice(wa, k):
                return wa[:, inner * k:inner * (k + 1)]

            masku8 = cpool.tile([128, NJ], u8, name="masku8")
            nc.sync.dma_start(
                out=masku8[:], in_=mask_d[:].rearrange("(c p) -> p c", p=128)
            )
            weng = nc.sync if dma_all_sync else nc.scalar
            qkeng = nc.scalar if wqk_scalar else weng
            xt_r = xt_d[:].rearrange("(k p) c -> p k c", p=128)
            for t in range(NI):
                ts = slice(512 * t, 512 * (t + 1))
                nc.sync.dma_start(
                    out=xTa[:].rearrange("p (k c) -> p k c", c=n)[:, :, ts],
                    in_=xt_r[:, :, ts],
                )
                if t == 0:
                    for wa, wd in ((wqa, wq_d), (wka, wk_d)):
                        qkeng.dma_start(
                            out=wa[:].rearrange("p (k c) -> p k c", c=inner),
                            in_=wd[:].rearrange("(k p) c -> p k c", p=128),
                        )
                if t == min(1, NI - 1):
                    weng.dma_start(
                        out=wva[:].rearrange("p (k c) -> p k c", c=inner),
                        in_=wv_d[:].rearrange("(k p) c -> p k c", p=128),
                    )
            for i in range(IC):
                weng.dma_start(out=wo[i][:], in_=wo_d[128 * i:128 * (i + 1), :])

            # PE warmup: dummy matmuls during the input-DMA wait ramp the
            # PE p-state before the first real matmul
            if warmup_mms:
                wup = cpool.tile([128, 512], bf, name="wup")
                nc.vector.memset(wup[:], 0.0)
                wps = mmpool.tile([128, 512], f32, tag="mm", name="wps")
                for i in range(warmup_mms):
                    nc.tensor.matmul(
                        wps[:], wup[:, 0:128], wup[:],
                        start=(i == 0), stop=(i == warmup_mms - 1),
                    )

            # bias[j] = (mask[j] - 1) * 1e30  ->  0 if kept, -1e30 if masked
            maskb = cpool.tile([128, NJ], f32, name="maskb")
            nc.vector.tensor_scalar(
                maskb[:], masku8[:], -1.0, 1e30, Alu.add, Alu.mult
            )
            # Schraudolph bias: B if kept; very negative if masked so the
            # fp32->int16 conversion saturates to 0x8000 = bf16 -0.0
            maskb16 = cpool.tile([128, NJ], f32, name="maskb16")
            nc.vector.tensor_scalar(
                maskb16[:], masku8[:], 1e6, SCH_B - 1e6, Alu.mult, Alu.add
            )

            QT = [cpool.tile([128, n], bf, name=f"QT{m}") for m in range(IC)]
            KT = [cpool.tile([128, n], bf, name=f"KT{m}") for m in range(IC)]
            # V pair tiles: [128 keys, 2 ktiles, heads * 80-col slots] fp8;
            # per head slot: cols 0..63 = V, col 64 = ones, rest zero
            V2 = [cpool.tile([128, 2 * heads * VS], f8, name=f"V2{c}")
                  for c in range(NP)]
            for c in range(NP):
                nc.vector.memset(V2[c][:], 0.0)
                nc.vector.memset(
                    V2[c][:].rearrange("p (i h s) -> p i h s", i=2, s=VS)
                    [:, :, :, dh:dh + 1],
                    1.0,
                )

            def v2r(c):
                return V2[c][:].rearrange("p (i c) -> p i c", i=2)

            AOT = [cpool.tile([128, n], bf, name=f"AOT{m}") for m in range(IC)]

            # ---- projections ----
            _proj_idx = [0]

            def _evac_on_act():
                _proj_idx[0] += 1
                return _proj_idx[0] <= early_act_evac

            def qk_proj_one(m, chain):
                W, OUT = ((wqa, QT), (wka, KT))[chain % 2]
                t = chain // 2
                ts = slice(512 * t, 512 * (t + 1))
                ps = otpool.tile([128, 512], f32, tag="ot", name="psqk")
                for k in range(KC):
                    nc.tensor.matmul(
                        ps[:],
                        wslice(W, k)[:, 128 * m:128 * (m + 1)],
                        xT(k)[:, ts],
                        start=(k == 0),
                        stop=(k == KC - 1),
                    )
                if _evac_on_act():
                    nc.scalar.activation(OUT[m][:, ts], ps[:], AF.Copy)
                else:
                    nc.vector.tensor_copy(OUT[m][:, ts], ps[:])

            def qk_proj(m):
                if qk_interleave:
                    for t in range(NI):
                        for chain in (0, 1):
                            qk_proj_one(m, 2 * t + chain)
                else:
                    for chain in (0, 1):
                        for t in range(NI):
                            qk_proj_one(m, 2 * t + chain)

            def v_proj(j):
                ps = otpool.tile([128, inner], f32, tag="ot", name="psv")
                for k in range(KC):
                    nc.tensor.matmul(
                        ps[:],
                        xT(k)[:, 128 * j:128 * (j + 1)],
                        wslice(wva, k),
                        start=(k == 0),
                        stop=(k == KC - 1),
                    )
                dst = (v2r(j // 2)[:, j % 2]
                       .rearrange("p (h s) -> p h s", s=VS)[:, :, 0:dh])
                nc.vector.tensor_copy(
                    dst, ps[:].rearrange("p (h v) -> p h v", v=dh)
                )

            qk_proj(0)
            if inject_v:
                v_proj(0)
                v_proj(1)
            else:
                for j in range(NJ):
                    v_proj(j)
            if not inject_qk1:
                for m in range(1, IC):
                    qk_proj(m)

            def final_proj(t):
                ps = mmpool.tile([128, d], f32, tag="mm", name="psf")
                for ic in range(IC):
                    nc.tensor.matmul(
                        ps[:],
                        AOT[ic][:, 128 * t:128 * (t + 1)],
                        wo[ic][:],
                        start=(ic == 0),
                        stop=(ic == IC - 1),
                    )
                ob = opool.tile([128, d], f32, tag="ob", name="ob")
                if t % 2 == 1:
                    nc.scalar.activation(ob[:], ps[:], AF.Copy)
                else:
                    nc.vector.tensor_copy(ob[:], ps[:])
                nc.sync.dma_start(out=out_d[128 * t:128 * (t + 1), :], in_=ob[:])

            # ---- attention; processed per key-chunk PAIR ----
            def attn_block(ih, pr, injections, ot=None):
                isl = slice(512 * ih, 512 * (ih + 1))
                if ot is None:
                    ot = otpool.tile([VW, 1024], f32, tag="ot", name="ot")
                for c in range(NP):
                    is_dve = c in dve_pairs
                    if is_dve:
                        pi = pipool.tile([128, 2048], i16, tag="pi", name="pi")
                        pbf = pi[:].bitcast(bf)
                    else:
                        p2 = p8pool.tile([128, 2048], f8, tag="p8", name="p8")
                        p2v = p2[:].rearrange("p (i c) -> p i c", i=2)
                    for half in range(2):
                        jc = 2 * c + half
                        jsl = slice(128 * jc, 128 * (jc + 1))
                        st = mmpool.tile([128, 1024], f32, tag="mm", name="st")
                        for hh in range(2):
                            rsl = slice(64 * hh, 64 * (hh + 1))
                            nc.tensor.matmul(
                                st[:, 512 * hh:512 * (hh + 1)],
                                KT[pr][rsl, jsl],
                                QT[pr][rsl, isl],
                                start=True,
                                stop=True,
                            )
                        if is_dve:
                            nc.vector.tensor_scalar(
                                pi[:, 1024 * half:1024 * (half + 1)],
                                st[:], SCH_AU * SCALE,
                                maskb16[:, jc:jc + 1],
                                Alu.mult, Alu.add,
                            )
                        else:
                            nc.scalar.activation(
                                p2[:, 1024 * half:1024 * (half + 1)],
                                st[:], AF.Exp,
                                bias=maskb[:, jc:jc + 1], scale=SCALE,
                            )
                        fn = injections.get(jc)
                        if fn is not None:
                            fn()
                    if dbg and pr == 0 and ih == 0 and c == 0 and not is_dve:
                        nc.sync.dma_start(out=dbg_p2[:], in_=p2[:])
                    first = (c == 0)
                    last = (c == NP - 1)
                    for hh in range(2):
                        h = 2 * pr + hh
                        osl = slice(512 * hh, 512 * (hh + 1))
                        if is_dve:
                            for half in range(2):
                                nc.tensor.matmul(
                                    ot[:, osl],
                                    v2r(c)[:, half, VS * h:VS * h + VW],
                                    pbf[:, 1024 * half + 512 * hh:
                                        1024 * half + 512 * (hh + 1)],
                                    start=(first and half == 0),
                                    stop=(last and half == 1),
                                )
                        elif use_dr:
                            nc.tensor.matmul(
                                ot[:, osl],
                                v2r(c)[:, :, VS * h:VS * h + VW],
                                p2v[:, :, 512 * hh:512 * (hh + 1)],
                                start=first,
                                stop=last,
                                perf_mode=DR,
                            )
                        else:
                            for half in range(2):
                                nc.tensor.matmul(
                                    ot[:, osl],
                                    v2r(c)[:, half, VS * h:VS * h + VW],
                                    p2v[:, half, 512 * hh:512 * (hh + 1)],
                                    start=(first and half == 0),
                                    stop=(last and half == 1),
                                )
                if dbg and pr == 0 and ih == 0:
                    otsb = cpool.tile([VW, 1024], f32, name="otsb_dbg")
                    nc.vector.tensor_copy(otsb[:], ot[:])
                    nc.sync.dma_start(out=dbg_ot[:], in_=otsb[:])
                # normalize: AOT rows = OT rows 0..dh-1 times 1/denom.
                for hh in range(2):
                    csl = slice(512 * hh, 512 * (hh + 1))
                    # reciprocal_approx_fast mis-addresses partition-offset
                    # PSUM inputs on HW; stage the denom strip to partition 0
                    # SBUF first (plain copy handles the partition shift).
                    rs = spool.tile([1, 512], f32, tag="rs", name="rs")
                    nc.vector.tensor_copy(rs[:], ot[dh:VW, csl])
                    rc = spool.tile([1, 512], f32, tag="rc", name="rc")
                    nc.vector.reciprocal_approx_fast(rc[:], rs[:])
                    if dbg and pr == 0 and ih == 0 and hh == 0:
                        nc.sync.dma_start(out=dbg_rc[:], in_=rc[:])
                    rcb = spool.tile([dh, 512], f32, tag="rcb", name="rcb")
                    nc.gpsimd.partition_broadcast(rcb[:], rc[:])
                    if hh == 0:
                        nc.vector.tensor_mul(
                            AOT[pr][0:dh, isl], ot[0:dh, csl], rcb[:]
                        )
                    else:
                        tb = spool.tile([dh, 512], bf, tag="tb", name="tb")
                        nc.vector.tensor_mul(tb[:], ot[0:dh, csl], rcb[:])
                        # gpsimd SWDGE: partition-shifting SBUF move off the
                        # engines' critical path
                        nc.gpsimd.dma_start(out=AOT[pr][64:128, isl], in_=tb[:])

            # pass 0 (QT/KT chunk 0): V projection rides in block ih=0,
            # QK chunk 1 projection rides in block ih=1.
            qk1_chains = ([(lambda c=c: qk_proj_one(1, c)) for c in range(2 * NI)]
                          if (IC > 1 and inject_qk1) else [])
            import contextlib
            prio_ctx = tc.high_priority if attn_prio else contextlib.nullcontext
            for ih in range(NI):
                inj = {}
                if ih == 0 and inject_v:
                    for jc in range(NJ - 2):
                        inj[jc] = (lambda j=jc + 2: v_proj(j))
                elif ih == 1 and inject_qk1 and NI >= 2:
                    for q, fn in enumerate(qk1_chains):
                        inj[2 * q] = fn
                with prio_ctx():
                    attn_block(ih, 0, inj, ot=ot0 if ih == 0 else None)
            if inject_qk1 and NI < 2:
                for m in range(1, IC):
                    qk_proj(m)

            # pass 1 (QT/KT chunk 1): output projection for query block ih-2
            # rides in block ih; the last two blocks' chunks drain at the end.
            for ih in range(NI):
                inj = {}
                if inject_final and ih >= 2:
                    for q in range(4):
                        inj[4 + 2 * q] = (lambda t=4 * (ih - 2) + q: final_proj(t))
                with prio_ctx():
                    attn_block(ih, IC - 1, inj)

            t0 = 4 * max(0, NI - 2) if inject_final else 0
            for t in range(t0, 4 * NI):
                final_proj(t)

            if dbg:
                nc.sync.dma_start(out=dbg_qt[:], in_=QT[0][:, 0:512])
                nc.sync.dma_start(out=dbg_kt[:], in_=KT[0][:, 0:512])
                nc.sync.dma_start(out=dbg_v2[:], in_=V2[0][:])
                nc.sync.dma_start(out=dbg_aot[:], in_=AOT[0][:, 0:512])

    nc.compile()
    return nc


_PROGRAM = None


def _get_program():
    global _PROGRAM
    if _PROGRAM is None:
        _PROGRAM = build_program()
    return _PROGRAM


def make_in_maps(x, mask, Wq, Wkv, Wout):
    """Host-side shard: slice + lay out the full inputs for each core.
    Matmul operands ship as bf16 (the same round-to-nearest-even a device
    cast would apply)."""
    import ml_dtypes

    bf16 = ml_dtypes.bfloat16
    in_maps = []
    for c in range(N_CORES):
        b, g = c // 2, c % 2
        cs = slice(INNER * g, INNER * (g + 1))
        vs = slice(D + INNER * g, D + INNER * (g + 1))
        in_maps.append({
            "xt": np.ascontiguousarray(x[b].T.astype(bf16)),
            "wq": np.ascontiguousarray(Wq[:, cs].astype(bf16)),
            "wk": np.ascontiguousarray(Wkv[:, cs].astype(bf16)),
            "wv": np.ascontiguousarray(Wkv[:, vs].astype(bf16)),
            "wo": np.ascontiguousarray(Wout[cs, :].astype(bf16)),
            "mask": np.ascontiguousarray(mask[b]).astype(np.uint8),
        })
    return in_maps


def combine_outputs(results, bout):
    """Host-side unshard: sum the two row-parallel partials per batch, add bias."""
    out = np.zeros((B, N, D), np.float32)
    for c in range(N_CORES):
        out[c // 2] += results[c]["out"]
    out += np.asarray(bout, np.float32)[None, None, :]
    return out


def kernel(**inputs):
    x = np.asarray(inputs["x"], np.float32)
    mask = np.asarray(inputs["mask"])
    Wq = np.asarray(inputs["Wq"], np.float32)
    Wkv = np.asarray(inputs["Wkv"], np.float32)
    Wout = np.asarray(inputs["Wout"], np.float32)
    bout = np.asarray(inputs["bout"], np.float32)

    from concourse.bass_utils import run_bass_kernel_spmd

    nc = _get_program()
    in_maps = make_in_maps(x, mask, Wq, Wkv, Wout)
    res = run_bass_kernel_spmd(nc, in_maps, list(range(N_CORES))).results
    return combine_outputs(res, bout)


if __name__ == "__main__":
    rng = np.random.default_rng(0)
    s = 1.0 / np.sqrt(D)
    demo = {
        "x": rng.standard_normal((B, N, D), np.float32),
        "mask": np.ones((B, N), bool),
        "Wq": rng.uniform(-s, s, (D, INNER * 2)).astype(np.float32),
        "Wkv": rng.uniform(-s, s, (D, INNER * 4)).astype(np.float32),
        "Wout": rng.uniform(-s, s, (INNER * 2, D)).astype(np.float32),
        "bout": rng.uniform(-s, s, D).astype(np.float32),
    }
    out = kernel(**demo)
    print("kernel output", out.shape, out.dtype, float(np.abs(out).max()))


# revision 19
# speedup vs baseline: 1.4011x; 1.4011x over previous
"""Trainium2 Bass kernel for nn_Attention_41472204210940.

Reference computation (per batch b):
    q = x @ Wq; k, v = split(x @ Wkv); multi-head attention (H=8, DH=64);
    out = attn_out @ Wout + bout.

Sharding over 8 NeuronCores: core c handles batch b = c//2 and head group
g = c%2 (heads 4g..4g+4). Each core emits a partial [2048, 512] output; the
host sums the two partials per batch and adds bout (row-parallel to_out).

Per-core device program:
  - QK^T in bf16 exactly as the tuned baseline: ST[j, i] = K^T Q per
    (head, key-chunk, query-block), softmax transposed so the mask bias is
    a per-partition scalar.
  - softmax exp is SPLIT between engines: most key-chunk pairs go through
    the ACT engine (exp -> fp8e4m3 output), a tunable subset through the
    DVE as a one-instruction Schraudolph exp (int16 = st*A + B, whose bits
    ARE the bf16 of exp; fp32->int16 saturation maps masked lanes to -0.0).
  - PV: for fp8 pairs one DoubleRow fp8 matmul contracts BOTH key chunks
    (K=2x128) in the cycles a bf16 matmul needs for one chunk (hardware
    measured: 260ns either way); V stored fp8 with a ones column per head
    (16B-aligned 80-col head slots per the s3_lw_dual_fp8 stride rule) so
    the softmax denominators fall out of the same matmul. Schraudolph
    pairs run per-chunk matmuls with fp8 stationary x bf16 moving (mixed
    dtypes measured exact on HW).
  - epilogue uses reciprocal_approx_fast (~5x cheaper than DVE reciprocal,
    18 bits) + gpsimd partition-broadcast + DVE multiply into AOT.
  - out[t] = sum_pair AOT_pair[:, t].T @ Wout_pair in bf16, as before.

fp8/Schraudolph noise budget (measured against the cached reference
inputs in numpy, element-exact): ~1.3-1.5e-2 vs the 2e-2 gate.
"""

import numpy as np

B, N, D = 4, 2048, 512
H_TOTAL, DH = 8, 64
HEADS = 4            # heads per core
INNER = HEADS * DH   # per-core inner width (256)
N_CORES = 8
SCALE = DH ** -0.5

# Schraudolph constants: int16(bits of bf16(exp(s))) ~= s*SCH_A + SCH_B
SCH_AU = 12102203.161561485 / 65536.0        # 2^23/ln2 / 2^16, unscaled
SCH_B = 1064866805.0 / 65536.0               # RMS-optimal bias /2^16


def build_program(n=N, d=D, heads=HEADS, dh=DH,
                  inject_v=True, inject_qk1=True, inject_final=True,
                  qk_interleave=True,
                  p8_bufs=6, pi_bufs=3, st_bufs=3,
                  attn_prio=True, dma_all_sync=True,
                  wqk_scalar=True,
                  early_act_evac=0, warmup_mms=24,
                  dve_pairs=(3, 6), use_dr=True, dbg=False):
    """Build + compile the per-core Bass program (SPMD; all cores run the
    identical program on different data).

    dve_pairs: pair indices (0..NJ/2-1) within every (pass, query-block)
    whose softmax runs on the DVE via Schraudolph instead of ACT exp."""
    import concourse.bacc as bacc
    import concourse.mybir as mybir
    from concourse import tile

    f32 = mybir.dt.float32
    bf = mybir.dt.bfloat16
    f8 = mybir.dt.float8e4
    i16 = mybir.dt.int16
    u8 = mybir.dt.uint8
    AF = mybir.ActivationFunctionType
    Alu = mybir.AluOpType
    DR = mybir.MatmulPerfMode.DoubleRow

    inner = heads * dh
    KC = d // 128          # k-chunks of the projection contraction dim
    IC = inner // 128      # 128-row chunks of QT/KT == head pairs
    NJ = n // 128          # key chunks
    NP = NJ // 2           # key chunk pairs
    NI = n // 512          # query blocks
    VW = dh + 1            # V columns per head incl. the ones column
    VS = 80                # V head slot width (16B-aligned for dual-fp8 LDW)

    assert dh == 64 and inner % 128 == 0 and n % 512 == 0 and d % 128 == 0

    nc = bacc.Bacc("TRN2", target_bir_lowering=False, debug=False)

    xt_d = nc.dram_tensor("xt", [d, n], bf, kind="ExternalInput")
    wq_d = nc.dram_tensor("wq", [d, inner], bf, kind="ExternalInput")
    wk_d = nc.dram_tensor("wk", [d, inner], bf, kind="ExternalInput")
    wv_d = nc.dram_tensor("wv", [d, inner], bf, kind="ExternalInput")
    wo_d = nc.dram_tensor("wo", [inner, d], bf, kind="ExternalInput")
    mask_d = nc.dram_tensor("mask", [n], u8, kind="ExternalInput")
    out_d = nc.dram_tensor("out", [n, d], f32, kind="ExternalOutput")
    if dbg:
        dbg_qt = nc.dram_tensor("dbg_qt", [128, 512], bf, kind="ExternalOutput")
        dbg_kt = nc.dram_tensor("dbg_kt", [128, 512], bf, kind="ExternalOutput")
        dbg_v2 = nc.dram_tensor("dbg_v2", [128, 640], f8, kind="ExternalOutput")
        dbg_p2 = nc.dram_tensor("dbg_p2", [128, 2048], f8, kind="ExternalOutput")
        dbg_ot = nc.dram_tensor("dbg_ot", [VW, 1024], f32, kind="ExternalOutput")
        dbg_rc = nc.dram_tensor("dbg_rc", [1, 512], f32, kind="ExternalOutput")
        dbg_aot = nc.dram_tensor("dbg_aot", [128, 512], bf, kind="ExternalOutput")

    with tile.TileContext(nc) as tc:
        with (
            nc.allow_low_precision(reason="bf16/fp8 matmul operand prep"),
            tc.tile_pool(name="const", bufs=1) as cpool,
            tc.tile_pool(name="p8w", bufs=p8_bufs) as p8pool,
            tc.tile_pool(name="piw", bufs=pi_bufs) as pipool,
            tc.tile_pool(name="small", bufs=2) as spool,
            tc.tile_pool(name="outsb", bufs=3) as opool,
            tc.tile_pool(name="mm", bufs=st_bufs, space="PSUM") as mmpool,
            tc.tile_pool(name="ot", bufs=2, space="PSUM") as otpool,
        ):
            # PSUM budget (16KB/partition): mm tag = st [128,1024] 4KB x
            # st_bufs; ot tag = per-head [65,512] 2KB x 2. All projection
            # psums share the mm pool so the ot pool is free at block 0.

            # ---- input loads: fused tiles, few DGE issues ----
            xTa = cpool.tile([128, KC * n], bf, name="xTa")
            wqa = cpool.tile([128, KC * inner], bf, name="wqa")
            wka = cpool.tile([128, KC * inner], bf, name="wka")
            wva = cpool.tile([128, KC * inner], bf, name="wva")
            wo = [cpool.tile([128, d], bf, name=f"wo{i}") for i in range(IC)]

            def xT(k):
                return xTa[:, n * k:n * (k + 1)]

            def wslice(wa, k):
                return wa[:, inner * k:inner * (k + 1)]

            masku8 = cpool.tile([128, NJ], u8, name="masku8")
            nc.sync.dma_start(
                out=masku8[:], in_=mask_d[:].rearrange("(c p) -> p c", p=128)
            )
            weng = nc.sync if dma_all_sync else nc.scalar
            qkeng = nc.scalar if wqk_scalar else weng
            xt_r = xt_d[:].rearrange("(k p) c -> p k c", p=128)
            for t in range(NI):
                ts = slice(512 * t, 512 * (t + 1))
                nc.sync.dma_start(
                    out=xTa[:].rearrange("p (k c) -> p k c", c=n)[:, :, ts],
                    in_=xt_r[:, :, ts],
                )
                if t == 0:
                    for wa, wd in ((wqa, wq_d), (wka, wk_d)):
                        qkeng.dma_start(
                            out=wa[:].rearrange("p (k c) -> p k c", c=inner),
                            in_=wd[:].rearrange("(k p) c -> p k c", p=128),
                        )
                if t == min(1, NI - 1):
                    weng.dma_start(
                        out=wva[:].rearrange("p (k c) -> p k c", c=inner),
                        in_=wv_d[:].rearrange("(k p) c -> p k c", p=128),
                    )
            for i in range(IC):
                weng.dma_start(out=wo[i][:], in_=wo_d[128 * i:128 * (i + 1), :])

            # PE warmup: dummy matmuls during the input-DMA wait ramp the
            # PE p-state before the first real matmul
            if warmup_mms:
                wup = cpool.tile([128, 512], bf, name="wup")
                nc.vector.memset(wup[:], 0.0)
                wps = mmpool.tile([128, 512], f32, tag="mm", name="wps")
                for i in range(warmup_mms):
                    nc.tensor.matmul(
                        wps[:], wup[:, 0:128], wup[:],
                        start=(i == 0), stop=(i == warmup_mms - 1),
                    )

            # bias[j] = (mask[j] - 1) * 1e30  ->  0 if kept, -1e30 if masked
            maskb = cpool.tile([128, NJ], f32, name="maskb")
            nc.vector.tensor_scalar(
                maskb[:], masku8[:], -1.0, 1e30, Alu.add, Alu.mult
            )
            # Schraudolph bias: B if kept; very negative if masked so the
            # fp32->int16 conversion saturates to 0x8000 = bf16 -0.0
            maskb16 = cpool.tile([128, NJ], f32, name="maskb16")
            nc.vector.tensor_scalar(
                maskb16[:], masku8[:], 1e6, SCH_B - 1e6, Alu.mult, Alu.add
            )

            QT = [cpool.tile([128, n], bf, name=f"QT{m}") for m in range(IC)]
            KT = [cpool.tile([128, n], bf, name=f"KT{m}") for m in range(IC)]
            # V pair tiles: [128 keys, 2 ktiles, heads * 80-col slots] fp8;
            # per head slot: cols 0..63 = V, col 64 = ones, rest zero
            V2 = [cpool.tile([128, 2 * heads * VS], f8, name=f"V2{c}")
                  for c in range(NP)]
            for c in range(NP):
                nc.vector.memset(V2[c][:], 0.0)
                nc.vector.memset(
                    V2[c][:].rearrange("p (i h s) -> p i h s", i=2, s=VS)
                    [:, :, :, dh:dh + 1],
                    1.0,
                )

            def v2r(c):
                return V2[c][:].rearrange("p (i c) -> p i c", i=2)

            AOT = [cpool.tile([128, n], bf, name=f"AOT{m}") for m in range(IC)]

            # ---- projections ----
            _proj_idx = [0]

            def _evac_on_act():
                _proj_idx[0] += 1
                return _proj_idx[0] <= early_act_evac

            def qk_proj_one(m, chain):
                W, OUT = ((wqa, QT), (wka, KT))[chain % 2]
                t = chain // 2
                ts = slice(512 * t, 512 * (t + 1))
                ps = mmpool.tile([128, 512], f32, tag="mm", name="psqk")
                for k in range(KC):
                    nc.tensor.matmul(
                        ps[:],
                        wslice(W, k)[:, 128 * m:128 * (m + 1)],
                        xT(k)[:, ts],
                        start=(k == 0),
                        stop=(k == KC - 1),
                    )
                if _evac_on_act():
                    nc.scalar.activation(OUT[m][:, ts], ps[:], AF.Copy)
                else:
                    nc.vector.tensor_copy(OUT[m][:, ts], ps[:])

            def qk_proj(m):
                if qk_interleave:
                    for t in range(NI):
                        for chain in (0, 1):
                            qk_proj_one(m, 2 * t + chain)
                else:
                    for chain in (0, 1):
                        for t in range(NI):
                            qk_proj_one(m, 2 * t + chain)

            def v_proj(j):
                ps = mmpool.tile([128, inner], f32, tag="mm", name="psv")
                for k in range(KC):
                    nc.tensor.matmul(
                        ps[:],
                        xT(k)[:, 128 * j:128 * (j + 1)],
                        wslice(wva, k),
                        start=(k == 0),
                        stop=(k == KC - 1),
                    )
                dst = (v2r(j // 2)[:, j % 2]
                       .rearrange("p (h s) -> p h s", s=VS)[:, :, 0:dh])
                nc.vector.tensor_copy(
                    dst, ps[:].rearrange("p (h v) -> p h v", v=dh)
                )

            qk_proj(0)
            if inject_v:
                v_proj(0)
                v_proj(1)
            else:
                for j in range(NJ):
                    v_proj(j)
            if not inject_qk1:
                for m in range(1, IC):
                    qk_proj(m)

            def final_proj(t):
                ps = mmpool.tile([128, d], f32, tag="mm", name="psf")
                for ic in range(IC):
                    nc.tensor.matmul(
                        ps[:],
                        AOT[ic][:, 128 * t:128 * (t + 1)],
                        wo[ic][:],
                        start=(ic == 0),
                        stop=(ic == IC - 1),
                    )
                ob = opool.tile([128, d], f32, tag="ob", name="ob")
                if t % 2 == 1:
                    nc.scalar.activation(ob[:], ps[:], AF.Copy)
                else:
                    nc.vector.tensor_copy(ob[:], ps[:])
                nc.sync.dma_start(out=out_d[128 * t:128 * (t + 1), :], in_=ob[:])

            # ---- attention; processed per key-chunk PAIR ----
            def attn_block(ih, pr, injections):
                isl = slice(512 * ih, 512 * (ih + 1))
                ots = [otpool.tile([VW, 512], f32, tag="ot", name=f"ot{hh}")
                       for hh in range(2)]
                for c in range(NP):
                    is_dve = c in dve_pairs
                    if is_dve:
                        pi = pipool.tile([128, 2048], i16, tag="pi", name="pi")
                        pbf = pi[:].bitcast(bf)
                    else:
                        p2 = p8pool.tile([128, 2048], f8, tag="p8", name="p8")
                        p2v = p2[:].rearrange("p (i c) -> p i c", i=2)
                    for half in range(2):
                        jc = 2 * c + half
                        jsl = slice(128 * jc, 128 * (jc + 1))
                        st = mmpool.tile([128, 1024], f32, tag="mm", name="st")
                        for hh in range(2):
                            rsl = slice(64 * hh, 64 * (hh + 1))
                            nc.tensor.matmul(
                                st[:, 512 * hh:512 * (hh + 1)],
                                KT[pr][rsl, jsl],
                                QT[pr][rsl, isl],
                                start=True,
                                stop=True,
                            )
                        if is_dve:
                            nc.vector.tensor_scalar(
                                pi[:, 1024 * half:1024 * (half + 1)],
                                st[:], SCH_AU * SCALE,
                                maskb16[:, jc:jc + 1],
                                Alu.mult, Alu.add,
                            )
                        else:
                            nc.scalar.activation(
                                p2[:, 1024 * half:1024 * (half + 1)],
                                st[:], AF.Exp,
                                bias=maskb[:, jc:jc + 1], scale=SCALE,
                            )
                        fn = injections.get(jc)
                        if fn is not None:
                            fn()
                    if dbg and pr == 0 and ih == 0 and c == 0 and not is_dve:
                        nc.sync.dma_start(out=dbg_p2[:], in_=p2[:])
                    first = (c == 0)
                    last = (c == NP - 1)
                    for hh in range(2):
                        h = 2 * pr + hh
                        if is_dve:
                            for half in range(2):
                                nc.tensor.matmul(
                                    ots[hh][:],
                                    v2r(c)[:, half, VS * h:VS * h + VW],
                                    pbf[:, 1024 * half + 512 * hh:
                                        1024 * half + 512 * (hh + 1)],
                                    start=(first and half == 0),
                                    stop=(last and half == 1),
                                )
                        elif use_dr:
                            nc.tensor.matmul(
                                ots[hh][:],
                                v2r(c)[:, :, VS * h:VS * h + VW],
                                p2v[:, :, 512 * hh:512 * (hh + 1)],
                                start=first,
                                stop=last,
                                perf_mode=DR,
                            )
                        else:
                            for half in range(2):
                                nc.tensor.matmul(
                                    ots[hh][:],
                                    v2r(c)[:, half, VS * h:VS * h + VW],
                                    p2v[:, half, 512 * hh:512 * (hh + 1)],
                                    start=(first and half == 0),
                                    stop=(last and half == 1),
                                )
                if dbg and pr == 0 and ih == 0:
                    otsb = cpool.tile([VW, 1024], f32, name="otsb_dbg")
                    nc.vector.tensor_copy(otsb[:, 0:512], ots[0][:])
                    nc.vector.tensor_copy(otsb[:, 512:1024], ots[1][:])
                    nc.sync.dma_start(out=dbg_ot[:], in_=otsb[:])
                # normalize: AOT rows = OT rows 0..dh-1 times 1/denom.
                for hh in range(2):
                    ot = ots[hh]
                    # reciprocal_approx_fast mis-addresses partition-offset
                    # PSUM inputs on HW; stage the denom strip to partition 0
                    # SBUF first (plain copy handles the partition shift).
                    rs = spool.tile([1, 512], f32, tag="rs", name="rs")
                    nc.vector.tensor_copy(rs[:], ot[dh:VW, :])
                    rc = spool.tile([1, 512], f32, tag="rc", name="rc")
                    nc.vector.reciprocal_approx_fast(rc[:], rs[:])
                    if dbg and pr == 0 and ih == 0 and hh == 0:
                        nc.sync.dma_start(out=dbg_rc[:], in_=rc[:])
                    rcb = spool.tile([dh, 512], f32, tag="rcb", name="rcb")
                    nc.gpsimd.partition_broadcast(rcb[:], rc[:])
                    if hh == 0:
                        nc.vector.tensor_mul(
                            AOT[pr][0:dh, isl], ot[0:dh, :], rcb[:]
                        )
                    else:
                        tb = spool.tile([dh, 512], bf, tag="tb", name="tb")
                        nc.vector.tensor_mul(tb[:], ot[0:dh, :], rcb[:])
                        # gpsimd SWDGE: partition-shifting SBUF move off the
                        # engines' critical path
                        nc.gpsimd.dma_start(out=AOT[pr][64:128, isl], in_=tb[:])

            # pass 0 (QT/KT chunk 0): V projection rides in block ih=0,
            # QK chunk 1 projection rides in block ih=1.
            qk1_chains = ([(lambda c=c: qk_proj_one(1, c)) for c in range(2 * NI)]
                          if (IC > 1 and inject_qk1) else [])
            import contextlib
            prio_ctx = tc.high_priority if attn_prio else contextlib.nullcontext
            for ih in range(NI):
                inj = {}
                if ih == 0 and inject_v:
                    for jc in range(NJ - 2):
                        inj[jc] = (lambda j=jc + 2: v_proj(j))
                elif ih == 1 and inject_qk1 and NI >= 2:
                    for q, fn in enumerate(qk1_chains):
                        inj[2 * q] = fn
                with prio_ctx():
                    attn_block(ih, 0, inj)
            if inject_qk1 and NI < 2:
                for m in range(1, IC):
                    qk_proj(m)

            # pass 1 (QT/KT chunk 1): output projection for query block ih-2
            # rides in block ih; the last two blocks' chunks drain at the end.
            for ih in range(NI):
                inj = {}
                if inject_final and ih >= 2:
                    for q in range(4):
                        inj[4 + 2 * q] = (lambda t=4 * (ih - 2) + q: final_proj(t))
                with prio_ctx():
                    attn_block(ih, IC - 1, inj)

            t0 = 4 * max(0, NI - 2) if inject_final else 0
            for t in range(t0, 4 * NI):
                final_proj(t)

            if dbg:
                nc.sync.dma_start(out=dbg_qt[:], in_=QT[0][:, 0:512])
                nc.sync.dma_start(out=dbg_kt[:], in_=KT[0][:, 0:512])
                nc.sync.dma_start(out=dbg_v2[:], in_=V2[0][:])
                nc.sync.dma_start(out=dbg_aot[:], in_=AOT[0][:, 0:512])

    nc.compile()
    return nc


_PROGRAM = None


def _get_program():
    global _PROGRAM
    if _PROGRAM is None:
        _PROGRAM = build_program()
    return _PROGRAM


def make_in_maps(x, mask, Wq, Wkv, Wout):
    """Host-side shard: slice + lay out the full inputs for each core.
    Matmul operands ship as bf16 (the same round-to-nearest-even a device
    cast would apply)."""
    import ml_dtypes

    bf16 = ml_dtypes.bfloat16
    in_maps = []
    for c in range(N_CORES):
        b, g = c // 2, c % 2
        cs = slice(INNER * g, INNER * (g + 1))
        vs = slice(D + INNER * g, D + INNER * (g + 1))
        in_maps.append({
            "xt": np.ascontiguousarray(x[b].T.astype(bf16)),
            "wq": np.ascontiguousarray(Wq[:, cs].astype(bf16)),
            "wk": np.ascontiguousarray(Wkv[:, cs].astype(bf16)),
            "wv": np.ascontiguousarray(Wkv[:, vs].astype(bf16)),
            "wo": np.ascontiguousarray(Wout[cs, :].astype(bf16)),
            "mask": np.ascontiguousarray(mask[b]).astype(np.uint8),
        })
    return in_maps


def combine_outputs(results, bout):
    """Host-side unshard: sum the two row-parallel partials per batch, add bias."""
    out = np.zeros((B, N, D), np.float32)
    for c in range(N_CORES):
        out[c // 2] += results[c]["out"]
    out += np.asarray(bout, np.float32)[None, None, :]
    return out


def kernel(**inputs):
    x = np.asarray(inputs["x"], np.float32)
    mask = np.asarray(inputs["mask"])
    Wq = np.asarray(inputs["Wq"], np.float32)
    Wkv = np.asarray(inputs["Wkv"], np.float32)
    Wout = np.asarray(inputs["Wout"], np.float32)
    bout = np.asarray(inputs["bout"], np.float32)

    from concourse.bass_utils import run_bass_kernel_spmd

    nc = _get_program()
    in_maps = make_in_maps(x, mask, Wq, Wkv, Wout)
    res = run_bass_kernel_spmd(nc, in_maps, list(range(N_CORES))).results
    return combine_outputs(res, bout)


if __name__ == "__main__":
    rng = np.random.default_rng(0)
    s = 1.0 / np.sqrt(D)
    demo = {
        "x": rng.standard_normal((B, N, D), np.float32),
        "mask": np.ones((B, N), bool),
        "Wq": rng.uniform(-s, s, (D, INNER * 2)).astype(np.float32),
        "Wkv": rng.uniform(-s, s, (D, INNER * 4)).astype(np.float32),
        "Wout": rng.uniform(-s, s, (INNER * 2, D)).astype(np.float32),
        "bout": rng.uniform(-s, s, D).astype(np.float32),
    }
    out = kernel(**demo)
    print("kernel output", out.shape, out.dtype, float(np.abs(out).max()))
